# revision 10
# baseline (speedup 1.0000x reference)
"""AttentionBlock (GroupNorm+SiLU -> qkv -> 8-head attn -> proj -> residual)
on 8 TRN2 NeuronCores, head-parallel.

Key structure: the torch-faithful reshape q.transpose(1,2).reshape(B*NH,N,d)
makes "head" h = spatial positions n in [512h, 512h+512) -- attention is
block-diagonal over spatial blocks, so each core independently computes the
full pipeline for its block of 512 spatial positions and emits the final
output columns out[:, 512h:512h+512].

Sequence-axis permutation freedom (attention is equivariant under a common
permutation of Q/K/V rows) lets us use t = chunk*512 + n' ordering
(chunk = c//64, n' = spatial), which makes every layout a cheap copy.

Perf structure:
- GroupNorm stats split across DVE (bn_stats, chunks 0/1/3) and ACT
  (Square/Copy with accum_out, chunk 2), fed by a bf16 copy of x
  (half the DMA) while the core's own f32 block loads in parallel.
- DMA priority: x chunks first on the sync ring; qkv weights trigger on
  the scalar ring only after the stats ACTs, so stats aren't starved.
- SiLU computed as u*sigmoid(u) = 0.5u(1+tanh(u/2)): tanh lives in the
  exp table set, so the whole kernel needs only 2 ACT table loads and
  the exp set is resident before attention starts.
- S-matmuls (K=64) and proj (K=64) run 2x via PE row tiling: tiles
  (0,0)/(64,0) process even/odd blocks concurrently.  K^T/Q live
  duplicated on both partition halves (two strided mirror DMAs);
  ONorm/proj weights use a parity split.
- Softmax denominators ride the O-matmul (ones row appended to V);
  the per-I reciprocal is pipelined one I behind so the PE never waits.
- Softmax skips max-subtraction (scores*scale within [-0.76, 0.86]).
"""

import sys

if "/opt/trn_rl_repo" not in sys.path:
    sys.path.append("/opt/trn_rl_repo")  # fallback; the axon-site copy wins

import numpy as np

import concourse.bacc as bacc
import concourse.tile as tile
from concourse import mybir
from concourse.bass_utils import run_bass_kernel_spmd

F32 = mybir.dt.float32
F32R = mybir.dt.float32r
BF16 = mybir.dt.bfloat16
AF = mybir.ActivationFunctionType
ALU = mybir.AluOpType

CH = 512          # channels
N = 4096          # spatial positions (64*64)
NB = 512          # spatial block per core
NCORES = 8
G = 32            # groups
GS = 16           # channels per group
EPS = 1e-5
SCALE = 0.125     # d ** -0.5, d = 64


def _build():
    nc = bacc.Bacc(None, target_bir_lowering=False)

    xfb = nc.declare_dram_parameter("xfb", [CH, N], BF16, isOutput=False)
    xblk = nc.declare_dram_parameter("xblk", [CH, NB], F32, isOutput=False)
    qkvwT = nc.declare_dram_parameter("qkvwT", [CH, 3 * CH], F32R, isOutput=False)
    qb = nc.declare_dram_parameter("qb", [128, 12], F32, isOutput=False)
    pw128 = nc.declare_dram_parameter("pw128", [128, 4 * CH], F32R, isOutput=False)
    pb = nc.declare_dram_parameter("pb", [128, 4], F32, isOutput=False)
    nw = nc.declare_dram_parameter("nw", [128, 4], F32, isOutput=False)
    nbias = nc.declare_dram_parameter("nbias", [128, 4], F32, isOutput=False)
    identb = nc.declare_dram_parameter("identb", [128, 128], BF16, isOutput=False)
    ones64 = nc.declare_dram_parameter("ones64", [1, 64], F32R, isOutput=False)
    sel8 = nc.declare_dram_parameter("sel8", [128, 8], F32, isOutput=False)
    selT = nc.declare_dram_parameter("selT", [8, 128], F32, isOutput=False)
    out = nc.declare_dram_parameter("out", [CH, NB], F32, isOutput=True)

    with tile.TileContext(nc) as tc:
        _emit(nc, tc, locals())
    nc.finalize()
    return nc


def _emit(nc, tc, P):
    from contextlib import ExitStack

    xfb, xblk, qkvwT, qb, pw128, pb = (P[k] for k in
        ("xfb", "xblk", "qkvwT", "qb", "pw128", "pb"))
    nw, nbias, identb, ones64, sel8, selT, out = (P[k] for k in
        ("nw", "nbias", "identb", "ones64", "sel8", "selT", "out"))

    with ExitStack() as es:
        # ---------- persistent pools ----------
        persist = es.enter_context(tc.tile_pool(name="persist", bufs=1))
        consts = es.enter_context(tc.tile_pool(name="consts", bufs=1))

        xblk_sb = persist.tile([128, 4 * NB], F32)          # [p, t*512+n']
        pw_sb = persist.tile([128, 4 * CH], F32R)           # parity-split proj w
        QT = persist.tile([128, N], F32R)                   # both halves, all chunks
        KT = persist.tile([128, N], F32R)
        Vp = persist.tile([128, 32 * 65], BF16)             # [V_j | ones]
        ONorm = persist.tile([128, N // 2], F32R)           # parity-split attn out

        qb_sb = consts.tile([128, 12], F32)
        pb_sb = consts.tile([128, 4], F32)
        nw_sb = consts.tile([128, 4], F32)
        nb_sb = consts.tile([128, 4], F32)
        id_sb = consts.tile([128, 128], BF16)
        ones64_sb = consts.tile([1, 64], F32R)
        sel8_sb = consts.tile([128, 8], F32)
        selT_sb = consts.tile([8, 128], F32)
        eps_sb = consts.tile([128, 1], F32)
        A_sb = consts.tile([128, 4], F32)
        B_sb = consts.tile([128, 4], F32)

        # ---------- phase B: loads + GroupNorm stats (DVE || ACT) ----------
        with ExitStack() as es_b, ExitStack() as es_cd:
            pools = es_b.enter_context(tc.tile_pool(name="pools", bufs=4))
            poolbig = es_b.enter_context(tc.tile_pool(name="poolbig", bufs=1))
            psA = es_b.enter_context(tc.tile_pool(name="psA", bufs=1, space="PSUM"))

            # stats source: bf16 full x, highest priority, on the sync ring
            xfb_sb = poolbig.tile([128, 4 * N], BF16, tag="xfb")
            for t in range(4):
                nc.sync.dma_start(out=xfb_sb[:, t * N:(t + 1) * N],
                                  in_=xfb[t * 128:(t + 1) * 128, :])
            # scalar ring program: xblk loads; then the chunk-2 stats ACTs;
            # qkv weights trigger only after those, keeping stats unstarved
            for t in range(4):
                nc.scalar.dma_start(out=xblk_sb[:, t * NB:(t + 1) * NB],
                                    in_=xblk[t * 128:(t + 1) * 128, :])
            # consts on the gpsimd queue
            nc.gpsimd.dma_start(out=sel8_sb[:], in_=sel8[:])
            nc.gpsimd.dma_start(out=selT_sb[:], in_=selT[:])
            nc.gpsimd.dma_start(out=nw_sb[:], in_=nw[:])
            nc.gpsimd.dma_start(out=nb_sb[:], in_=nbias[:])
            nc.gpsimd.dma_start(out=qb_sb[:], in_=qb[:])
            nc.gpsimd.dma_start(out=pb_sb[:], in_=pb[:])
            nc.gpsimd.dma_start(out=id_sb[:], in_=identb[:])
            nc.gpsimd.dma_start(out=ones64_sb[:], in_=ones64[:])
            nc.vector.memset(eps_sb[:], EPS)

            # pin the ACT table set to sqrt_and_others (square/copy are in
            # every set) so stats + finalize need one load total
            junk1 = pools.tile([128, 1], F32, tag="junk1")
            nc.scalar.activation(out=junk1[:], in_=eps_sb[:], func=AF.Sqrt)

            chs = pools.tile([128, 8], F32, tag="chs")      # [mean_t, ex2_t]*4
            # chunk 2 on ACT (Square/Copy with accumulate)
            sq = poolbig.tile([128, N], BF16, tag="sq")
            a2 = pools.tile([128, 1], F32, tag="a2")
            a1 = pools.tile([128, 1], F32, tag="a1")
            nc.scalar.activation(out=sq[:], in_=xfb_sb[:, 2 * N:3 * N],
                                 func=AF.Square, accum_out=a2[:])
            nc.scalar.activation(out=sq[:], in_=xfb_sb[:, 2 * N:3 * N],
                                 func=AF.Copy, accum_out=a1[:])
            nc.vector.tensor_scalar_mul(chs[:, 4:5], a1[:], 1.0 / N)
            nc.vector.tensor_scalar_mul(chs[:, 5:6], a2[:], 1.0 / N)
            # weights now: qkv first, proj after
            qkvw_sb = es_cd.enter_context(
                tc.tile_pool(name="poolq", bufs=1)).tile([128, 4 * 1536], F32R)
            for kt in range(4):
                nc.scalar.dma_start(out=qkvw_sb[:, kt * 1536:(kt + 1) * 1536],
                                    in_=qkvwT[kt * 128:(kt + 1) * 128, :])
            nc.scalar.dma_start(out=pw_sb[:], in_=pw128[:])

            # chunks 0, 1, 3 on DVE bn_stats
            for t in (0, 1, 3):
                st = pools.tile([128, 8, 6], F32, tag="st")
                for k in range(8):
                    nc.vector.bn_stats(out=st[:, k, :],
                                       in_=xfb_sb[:, t * N + k * 512:
                                                  t * N + (k + 1) * 512])
                mv = pools.tile([128, 2], F32, tag="mv")
                nc.vector.bn_aggr(out=mv[:], in_=st[:])
                nc.vector.tensor_copy(chs[:, 2 * t:2 * t + 1], mv[:, 0:1])
                msq = pools.tile([128, 1], F32, tag="msq")
                nc.vector.tensor_tensor(out=msq[:], in0=mv[:, 0:1], in1=mv[:, 0:1],
                                        op=ALU.mult)
                nc.vector.tensor_tensor(out=chs[:, 2 * t + 1:2 * t + 2],
                                        in0=msq[:], in1=mv[:, 1:2],
                                        op=ALU.add)

            # group fold: per-channel -> per-group (sel8 = 1/16), then back
            gp = psA.tile([8, 8], F32, tag="gp")
            for t in range(4):
                nc.tensor.matmul(gp[:, 2 * t:2 * t + 2], lhsT=sel8_sb[:],
                                 rhs=chs[:, 2 * t:2 * t + 2], start=True, stop=True)
            gp_sb = pools.tile([8, 8], F32, tag="gpsb")
            nc.vector.tensor_scalar_mul(gp_sb[:], gp[:], float(NCORES))
            gx = psA.tile([128, 8], F32, tag="gx")
            for t in range(4):
                nc.tensor.matmul(gx[:, 2 * t:2 * t + 2], lhsT=selT_sb[:],
                                 rhs=gp_sb[:, 2 * t:2 * t + 2], start=True, stop=True)
            gxs = pools.tile([128, 8], F32, tag="gxs")
            nc.vector.tensor_copy(gxs[:], gx[:])
            gx3 = gxs.rearrange("p (t two) -> p t two", two=2)
            musq = pools.tile([128, 4], F32, tag="musq")
            nc.vector.tensor_tensor(out=musq[:], in0=gx3[:, :, 0], in1=gx3[:, :, 0],
                                    op=ALU.mult)
            var = pools.tile([128, 4], F32, tag="var")
            nc.vector.tensor_tensor(out=var[:], in0=gx3[:, :, 1], in1=musq[:],
                                    op=ALU.subtract)
            sd = pools.tile([128, 4], F32, tag="sd")
            nc.scalar.activation(out=sd[:], in_=var[:], func=AF.Sqrt,
                                 bias=eps_sb[:], scale=1.0)
            rstd = pools.tile([128, 4], F32, tag="rstd")
            nc.vector.reciprocal(out=rstd[:], in_=sd[:])
            nc.vector.tensor_tensor(out=A_sb[:], in0=rstd[:], in1=nw_sb[:],
                                    op=ALU.mult)
            muA = pools.tile([128, 4], F32, tag="muA")
            nc.vector.tensor_tensor(out=muA[:], in0=gx3[:, :, 0], in1=A_sb[:],
                                    op=ALU.mult)
            nc.vector.tensor_tensor(out=B_sb[:], in0=nb_sb[:], in1=muA[:],
                                    op=ALU.subtract)

            # ---------- phase C: normalize + SiLU(tanh form) + qkv ----------
            poolq2 = es_cd.enter_context(tc.tile_pool(name="poolq2", bufs=1))
            psB = es_b.enter_context(tc.tile_pool(name="psB", bufs=3, space="PSUM"))

            # u = A*x+B; sig = 0.5 + 0.5*tanh(u/2) == sigmoid(u); h = u*sig.
            # tanh is in exp_and_others: this also pre-loads the exp table.
            u_sb = poolq2.tile([128, 2048], F32)
            t_sb = poolq2.tile([128, 2048], F32)
            s_sb = poolq2.tile([128, 2048], F32)
            h_sb = poolq2.tile([128, 2048], F32R)
            for t in range(4):
                csl = slice(t * 512, (t + 1) * 512)
                eng = nc.vector if t % 2 == 0 else nc.gpsimd
                eng.tensor_scalar(out=u_sb[:, csl], in0=xblk_sb[:, csl],
                                  scalar1=A_sb[:, t:t + 1],
                                  scalar2=B_sb[:, t:t + 1],
                                  op0=ALU.mult, op1=ALU.add)
                nc.scalar.activation(out=t_sb[:, csl], in_=u_sb[:, csl],
                                     func=AF.Tanh, scale=0.5)
                eng.tensor_scalar(out=s_sb[:, csl], in0=t_sb[:, csl],
                                  scalar1=0.5, scalar2=0.5,
                                  op0=ALU.mult, op1=ALU.add)
                eng.tensor_tensor(out=h_sb[:, csl], in0=u_sb[:, csl],
                                  in1=s_sb[:, csl], op=ALU.mult)

            vs = poolq2.tile([128, 2048], BF16)
            # qkv order: k chunks, v chunks, then q -- K/Q mirrors + V
            # transposes all complete by the time S needs them.
            # q/k chunk 2t -> rows 0:64 cols 2t*512; 2t+1 -> rows 64:128.
            for ot in [4, 5, 6, 7, 8, 9, 10, 11, 0, 1, 2, 3]:
                ps = psB.tile([128, 512], F32, tag="qkvps")
                for kt in range(4):
                    nc.tensor.matmul(
                        ps[:],
                        lhsT=qkvw_sb[:, kt * 1536 + ot * 128:
                                     kt * 1536 + (ot + 1) * 128],
                        rhs=h_sb[:, kt * 512:(kt + 1) * 512],
                        start=(kt == 0), stop=(kt == 3))
                kind, t = ot // 4, ot % 4
                if kind == 2:
                    nc.vector.tensor_scalar_add(vs[:, t * 512:(t + 1) * 512], ps[:],
                                                qb_sb[:, ot:ot + 1])
                else:
                    dst = QT if kind == 0 else KT
                    nc.vector.tensor_scalar_add(
                        dst[0:64, (2 * t) * 512:(2 * t + 1) * 512],
                        ps[0:64, :], qb_sb[0:64, ot:ot + 1])
                    nc.vector.tensor_scalar_add(
                        dst[64:128, (2 * t + 1) * 512:(2 * t + 2) * 512],
                        ps[64:128, :], qb_sb[64:128, ot:ot + 1])
                if ot == 7 or ot == 3:
                    # mirror: even chunks (rows 0:64) up, odd chunks down,
                    # so both partition halves hold every chunk
                    dst = KT if ot == 7 else QT
                    d3 = dst.rearrange("p (u two n) -> p u two n", two=2, n=512)
                    nc.sync.dma_start(out=d3[64:128, :, 0, :], in_=d3[0:64, :, 0, :])
                    nc.sync.dma_start(out=d3[0:64, :, 1, :], in_=d3[64:128, :, 1, :])
                if ot == 11:
                    # ---------- phase D: Vp layout (PE transposes, bf16) ----
                    Vp3 = Vp.rearrange("p (j c) -> p j c", c=65)
                    nc.vector.memset(Vp3[:, :, 64:65], 1.0)
                    for tt in range(4):
                        for b in range(4):
                            pst = psB.tile([128, 128], BF16, tag="vtr")
                            nc.tensor.transpose(
                                pst[:],
                                in_=vs[:, tt * 512 + b * 128:tt * 512 + (b + 1) * 128],
                                identity=id_sb[:])
                            j1, j2 = 8 * tt + b, 8 * tt + 4 + b
                            nc.vector.tensor_copy(Vp3[:, j1, 0:64], pst[:, 0:64])
                            nc.vector.tensor_copy(Vp3[:, j2, 0:64], pst[:, 64:128])

        # ---------- phase E: attention (software-pipelined S/exp | O) ----------
        with ExitStack() as es_e:
            psS = es_e.enter_context(tc.tile_pool(name="psS", bufs=2, space="PSUM"))
            psO = es_e.enter_context(tc.tile_pool(name="psO", bufs=2, space="PSUM"))
            poolPB = es_e.enter_context(tc.tile_pool(name="poolPB", bufs=2))
            poolsm = es_e.enter_context(tc.tile_pool(name="poolsm", bufs=5))

            groups = [(j0, min(3, 32 - j0)) for j0 in range(0, 32, 3)]
            Vp3 = Vp.rearrange("p (j c) -> p j c", c=65)
            PBts, opss, OuSs, rDs = {}, {}, {}, {}

            def emit_drain_a(I):
                # evacuate O sums + reciprocal of the denominator row; the
                # dependent broadcast matmul runs one I later (emit_drain_b)
                # so the PE never waits on the reciprocal
                OuSs[I] = poolsm.tile([65, 512], F32, tag="OuS", name=f"OuS{I}")
                nc.vector.tensor_copy(OuSs[I][:], opss[I][:])
                rDs[I] = poolsm.tile([1, 512], F32R, tag="rD", name=f"rD{I}")
                with nc.allow_low_precision(reason="f32r output is f32 bits"):
                    nc.vector.reciprocal(out=rDs[I][:], in_=OuSs[I][64:65, :])
                del opss[I], PBts[I]

            def emit_drain_b(I):
                u, par = I // 2, I % 2
                dps = psO.tile([64, 512], F32, tag="ops", name=f"dps{I}")
                nc.tensor.matmul(dps[:], lhsT=ones64_sb[:],
                                 rhs=rDs[I][:], start=True, stop=True)
                if par == 0:
                    nc.vector.tensor_tensor(out=ONorm[0:64, u * 512:(u + 1) * 512],
                                            in0=OuSs[I][0:64, :], in1=dps[:],
                                            op=ALU.mult)
                else:
                    stg = poolsm.tile([64, 512], F32, tag="stg", name=f"stg{I}")
                    nc.vector.tensor_tensor(out=stg[:], in0=OuSs[I][0:64, :],
                                            in1=dps[:], op=ALU.mult)
                    nc.gpsimd.dma_start(out=ONorm[64:128, u * 512:(u + 1) * 512],
                                        in_=stg[:])
                del OuSs[I], rDs[I]

            for I in range(9):
                if I < 8:
                    isl = slice(I * 512, (I + 1) * 512)
                    PBts[I] = poolPB.tile([128, 32 * 512], BF16, tag="PBt",
                                          name=f"PBt{I}")
                    opss[I] = psO.tile([65, 512], F32, tag="ops", name=f"ops{I}")
                for (j0, glen) in groups:
                    if I < 8:
                        sp = psS.tile([128, 1536], F32, tag="sp")
                        for jj in range(glen):
                            j = j0 + jj
                            r = slice(64, 128) if j % 2 else slice(0, 64)
                            nc.tensor.matmul(
                                sp[:, jj * 512:(jj + 1) * 512],
                                lhsT=KT[r, j * 128:(j + 1) * 128],
                                rhs=QT[r, isl],
                                start=True, stop=True)
                        nc.scalar.activation(
                            out=PBts[I][:, j0 * 512:(j0 + glen) * 512],
                            in_=sp[:, 0:glen * 512], func=AF.Exp, scale=SCALE)
                    if I > 0:
                        for jj in range(glen):
                            j = j0 + jj
                            nc.tensor.matmul(opss[I - 1][:], lhsT=Vp3[:, j, 0:65],
                                             rhs=PBts[I - 1][:, j * 512:(j + 1) * 512],
                                             start=(j == 0), stop=(j == 31))
                if I > 0:
                    emit_drain_a(I - 1)
                if I > 1:
                    emit_drain_b(I - 2)
            emit_drain_b(7)

        # ---------- phase F: proj (row-tiled) + bias + residual ----------
        with ExitStack() as es_f:
            psP = es_f.enter_context(tc.tile_pool(name="psP", bufs=4, space="PSUM"))
            poolf = es_f.enter_context(tc.tile_pool(name="poolf", bufs=2))
            for ot in range(4):
                ppA = psP.tile([128, 512], F32, tag="ppA")
                ppB = psP.tile([128, 512], F32, tag="ppB")
                for u in range(4):
                    nc.tensor.matmul(
                        ppA[:],
                        lhsT=pw_sb[0:64, u * 512 + ot * 128:u * 512 + (ot + 1) * 128],
                        rhs=ONorm[0:64, u * 512:(u + 1) * 512],
                        start=(u == 0), stop=(u == 3))
                for u in range(4):
                    nc.tensor.matmul(
                        ppB[:],
                        lhsT=pw_sb[64:128, u * 512 + ot * 128:u * 512 + (ot + 1) * 128],
                        rhs=ONorm[64:128, u * 512:(u + 1) * 512],
                        start=(u == 0), stop=(u == 3))
                fin = poolf.tile([128, 512], F32, tag="fin")
                nc.vector.tensor_scalar_add(fin[:], ppA[:], pb_sb[:, ot:ot + 1])
                nc.vector.tensor_tensor(out=fin[:], in0=fin[:], in1=ppB[:],
                                        op=ALU.add)
                nc.vector.tensor_tensor(out=fin[:], in0=fin[:],
                                        in1=xblk_sb[:, ot * 512:(ot + 1) * 512],
                                        op=ALU.add)
                nc.sync.dma_start(out=out[ot * 128:(ot + 1) * 128, :], in_=fin[:])


def _host_inputs(x, norm_w, norm_b, qkv_w, qkv_b, proj_w, proj_b):
    import ml_dtypes
    x2d = np.ascontiguousarray(np.asarray(x, np.float32).reshape(CH, N))
    qkv_w = np.asarray(qkv_w, np.float32)
    proj_w = np.asarray(proj_w, np.float32)
    # parity-split proj weights: chunk c -> rows 64*(c%2), cols (c//2)*512+o
    pw3 = proj_w.reshape(CH, 8, 64)          # [o, chunk, d']
    pw128 = np.zeros((128, 4 * CH), np.float32)
    for c in range(8):
        pw128[(c % 2) * 64:(c % 2) * 64 + 64, (c // 2) * CH:(c // 2 + 1) * CH] = \
            pw3[:, c, :].T
    common = {
        "xfb": np.ascontiguousarray(x2d.astype(ml_dtypes.bfloat16)),
        "qkvwT": np.ascontiguousarray(qkv_w.T),
        "qb": np.ascontiguousarray(np.asarray(qkv_b, np.float32).reshape(12, 128).T),
        "pw128": np.ascontiguousarray(pw128),
        "pb": np.ascontiguousarray(np.asarray(proj_b, np.float32).reshape(4, 128).T),
        "nw": np.ascontiguousarray(np.asarray(norm_w, np.float32).reshape(4, 128).T),
        "nbias": np.ascontiguousarray(np.asarray(norm_b, np.float32).reshape(4, 128).T),
        "identb": np.eye(128, dtype=ml_dtypes.bfloat16),
        "ones64": np.ones((1, 64), np.float32),
        "sel8": np.ascontiguousarray(
            (np.arange(128)[:, None] // GS == np.arange(8)[None, :])
            .astype(np.float32) / GS),
        "selT": np.ascontiguousarray(
            (np.arange(128)[None, :] // GS == np.arange(8)[:, None])
            .astype(np.float32) / NCORES),
    }
    in_maps = []
    for h in range(NCORES):
        m = dict(common)
        m["xblk"] = np.ascontiguousarray(x2d[:, h * NB:(h + 1) * NB])
        in_maps.append(m)
    return in_maps


_LAST_RESULT = {}


def kernel(x, norm_w, norm_b, qkv_w, qkv_b, proj_w, proj_b, _trace=False,
           _tmpdir=None):
    nc = _build()
    in_maps = _host_inputs(x, norm_w, norm_b, qkv_w, qkv_b, proj_w, proj_b)
    res = run_bass_kernel_spmd(nc, in_maps, core_ids=list(range(NCORES)),
                               trace=_trace, tmpdir=_tmpdir)
    _LAST_RESULT["res"] = res
    full = np.concatenate([res.results[h]["out"] for h in range(NCORES)], axis=1)
    return full.reshape(1, CH, 64, 64).astype(np.float32)


# revision 16
# speedup vs baseline: 1.0563x; 1.0563x over previous
"""AttentionBlock (GroupNorm+SiLU -> qkv -> 8-head attn -> proj -> residual)
on 8 TRN2 NeuronCores, head-parallel.

Key structure: the torch-faithful reshape q.transpose(1,2).reshape(B*NH,N,d)
makes "head" h = spatial positions n in [512h, 512h+512) -- attention is
block-diagonal over spatial blocks, so each core independently computes the
full pipeline for its block of 512 spatial positions and emits the final
output columns out[:, 512h:512h+512].

Sequence-axis permutation freedom (attention is equivariant under a common
permutation of Q/K/V rows) lets us use t = chunk*512 + n' ordering
(chunk = c//64, n' = spatial), which makes every layout a cheap copy.

Perf structure:
- GroupNorm stats split across DVE (bn_stats, chunks 0/1/3) and ACT
  (Square/Copy with accum_out, chunk 2), fed by a bf16 copy of x
  (half the DMA) while the core's own f32 block loads in parallel.
- DMA priority: x chunks first on the sync ring; qkv weights trigger on
  the scalar ring only after the stats ACTs, so stats aren't starved.
- SiLU computed as u*sigmoid(u) = 0.5u(1+tanh(u/2)): tanh lives in the
  exp table set, so the whole kernel needs only 2 ACT table loads and
  the exp set is resident before attention starts.
- S-matmuls (K=64) and proj (K=64) run 2x via PE row tiling: tiles
  (0,0)/(64,0) process even/odd blocks concurrently.  K^T/Q live
  duplicated on both partition halves (two strided mirror DMAs);
  ONorm/proj weights use a parity split.
- Softmax denominators ride the O-matmul (ones row appended to V);
  the per-I reciprocal is pipelined one I behind so the PE never waits.
- Softmax skips max-subtraction (scores*scale within [-0.76, 0.86]).
"""

import sys

if "/opt/trn_rl_repo" not in sys.path:
    sys.path.append("/opt/trn_rl_repo")  # fallback; the axon-site copy wins

import numpy as np

import concourse.bacc as bacc
import concourse.tile as tile
from concourse import mybir
from concourse.bass_utils import run_bass_kernel_spmd

F32 = mybir.dt.float32
F32R = mybir.dt.float32r
BF16 = mybir.dt.bfloat16
AF = mybir.ActivationFunctionType
ALU = mybir.AluOpType

CH = 512          # channels
N = 4096          # spatial positions (64*64)
NB = 512          # spatial block per core
NCORES = 8
G = 32            # groups
GS = 16           # channels per group
EPS = 1e-5
SCALE = 0.125     # d ** -0.5, d = 64


def _build():
    nc = bacc.Bacc(None, target_bir_lowering=False)

    xfb = nc.declare_dram_parameter("xfb", [CH, N], BF16, isOutput=False)
    xblk = nc.declare_dram_parameter("xblk", [CH, NB], F32, isOutput=False)
    qkvwT = nc.declare_dram_parameter("qkvwT", [CH, 3 * CH], F32R, isOutput=False)
    qb = nc.declare_dram_parameter("qb", [128, 12], F32, isOutput=False)
    pw128 = nc.declare_dram_parameter("pw128", [128, 4 * CH], F32R, isOutput=False)
    pb = nc.declare_dram_parameter("pb", [128, 4], F32, isOutput=False)
    nw = nc.declare_dram_parameter("nw", [128, 4], F32, isOutput=False)
    nbias = nc.declare_dram_parameter("nbias", [128, 4], F32, isOutput=False)
    identb = nc.declare_dram_parameter("identb", [128, 128], BF16, isOutput=False)
    ones64 = nc.declare_dram_parameter("ones64", [1, 64], F32R, isOutput=False)
    sel8 = nc.declare_dram_parameter("sel8", [128, 8], F32, isOutput=False)
    selT = nc.declare_dram_parameter("selT", [8, 128], F32, isOutput=False)
    out = nc.declare_dram_parameter("out", [CH, NB], F32, isOutput=True)

    with tile.TileContext(nc) as tc:
        _emit(nc, tc, locals())
    nc.finalize()
    return nc


def _emit(nc, tc, P):
    from contextlib import ExitStack

    xfb, xblk, qkvwT, qb, pw128, pb = (P[k] for k in
        ("xfb", "xblk", "qkvwT", "qb", "pw128", "pb"))
    nw, nbias, identb, ones64, sel8, selT, out = (P[k] for k in
        ("nw", "nbias", "identb", "ones64", "sel8", "selT", "out"))

    with ExitStack() as es:
        # ---------- persistent pools ----------
        persist = es.enter_context(tc.tile_pool(name="persist", bufs=1))
        consts = es.enter_context(tc.tile_pool(name="consts", bufs=1))

        xblk_sb = persist.tile([128, 4 * NB], F32)          # [p, t*512+n']
        pw_sb = persist.tile([128, 4 * CH], F32R)           # parity-split proj w
        QT = persist.tile([128, N], F32R)                   # both halves, all chunks
        KT = persist.tile([128, N], F32R)
        Vp = persist.tile([128, 32 * 65], BF16)             # [V_j | ones]
        ONorm = persist.tile([128, N // 2], F32R)           # parity-split attn out

        qb_sb = consts.tile([128, 12], F32)
        pb_sb = consts.tile([128, 4], F32)
        nw_sb = consts.tile([128, 4], F32)
        nb_sb = consts.tile([128, 4], F32)
        id_sb = consts.tile([128, 128], BF16)
        ones64_sb = consts.tile([1, 64], F32R)
        sel8_sb = consts.tile([128, 8], F32)
        selT_sb = consts.tile([8, 128], F32)
        eps_sb = consts.tile([128, 1], F32)
        A_sb = consts.tile([128, 4], F32)
        B_sb = consts.tile([128, 4], F32)

        # ---------- phase B: loads + GroupNorm stats (DVE || ACT) ----------
        with ExitStack() as es_b, ExitStack() as es_cd:
            pools = es_b.enter_context(tc.tile_pool(name="pools", bufs=4))
            poolbig = es_b.enter_context(tc.tile_pool(name="poolbig", bufs=1))
            psA = es_b.enter_context(tc.tile_pool(name="psA", bufs=1, space="PSUM"))

            # stats source: bf16 full x, highest priority, split across BOTH
            # hwdge rings so all four chunks land early
            xfb_sb = poolbig.tile([128, 4 * N], BF16, tag="xfb")
            for t in (0, 2):
                nc.sync.dma_start(out=xfb_sb[:, t * N:(t + 1) * N],
                                  in_=xfb[t * 128:(t + 1) * 128, :])
            for t in (1, 3):
                nc.scalar.dma_start(out=xfb_sb[:, t * N:(t + 1) * N],
                                    in_=xfb[t * 128:(t + 1) * 128, :])
            for t in range(4):
                nc.scalar.dma_start(out=xblk_sb[:, t * NB:(t + 1) * NB],
                                    in_=xblk[t * 128:(t + 1) * 128, :])
            # consts on the gpsimd queue
            nc.gpsimd.dma_start(out=sel8_sb[:], in_=sel8[:])
            nc.gpsimd.dma_start(out=selT_sb[:], in_=selT[:])
            nc.gpsimd.dma_start(out=nw_sb[:], in_=nw[:])
            nc.gpsimd.dma_start(out=nb_sb[:], in_=nbias[:])
            nc.gpsimd.dma_start(out=qb_sb[:], in_=qb[:])
            nc.gpsimd.dma_start(out=pb_sb[:], in_=pb[:])
            nc.gpsimd.dma_start(out=id_sb[:], in_=identb[:])
            nc.gpsimd.dma_start(out=ones64_sb[:], in_=ones64[:])
            nc.vector.memset(eps_sb[:], EPS)

            # pin the ACT table set to sqrt_and_others (square/copy are in
            # every set) so stats + finalize need one load total
            junk1 = pools.tile([128, 1], F32, tag="junk1")
            nc.scalar.activation(out=junk1[:], in_=eps_sb[:], func=AF.Sqrt)

            chs = pools.tile([128, 8], F32, tag="chs")      # [mean_t, ex2_t]*4
            # chunk 2 on ACT (Square/Copy with accumulate)
            sq = poolbig.tile([128, N], BF16, tag="sq")
            a2 = pools.tile([128, 1], F32, tag="a2")
            a1 = pools.tile([128, 1], F32, tag="a1")
            nc.scalar.activation(out=sq[:], in_=xfb_sb[:, 2 * N:3 * N],
                                 func=AF.Square, accum_out=a2[:])
            # qkv weights trigger here: after the stats loads, so they don't
            # starve them, but early enough to land before the qkv matmuls
            qkvw_sb = es_cd.enter_context(
                tc.tile_pool(name="poolq", bufs=1)).tile([128, 4 * 1536], F32R)
            for kt in range(4):
                nc.scalar.dma_start(out=qkvw_sb[:, kt * 1536:(kt + 1) * 1536],
                                    in_=qkvwT[kt * 128:(kt + 1) * 128, :])
            nc.scalar.activation(out=sq[:], in_=xfb_sb[:, 2 * N:3 * N],
                                 func=AF.Copy, accum_out=a1[:])
            nc.scalar.dma_start(out=pw_sb[:], in_=pw128[:])
            nc.vector.tensor_scalar_mul(chs[:, 4:5], a1[:], 1.0 / N)
            nc.vector.tensor_scalar_mul(chs[:, 5:6], a2[:], 1.0 / N)

            # chunks 0, 1, 3 on DVE bn_stats
            for t in (0, 1, 3):
                st = pools.tile([128, 8, 6], F32, tag="st")
                for k in range(8):
                    nc.vector.bn_stats(out=st[:, k, :],
                                       in_=xfb_sb[:, t * N + k * 512:
                                                  t * N + (k + 1) * 512])
                mv = pools.tile([128, 2], F32, tag="mv")
                nc.vector.bn_aggr(out=mv[:], in_=st[:])
                nc.vector.tensor_copy(chs[:, 2 * t:2 * t + 1], mv[:, 0:1])
                msq = pools.tile([128, 1], F32, tag="msq")
                nc.vector.tensor_tensor(out=msq[:], in0=mv[:, 0:1], in1=mv[:, 0:1],
                                        op=ALU.mult)
                nc.vector.tensor_tensor(out=chs[:, 2 * t + 1:2 * t + 2],
                                        in0=msq[:], in1=mv[:, 1:2],
                                        op=ALU.add)

            # group fold: per-channel -> per-group (sel8 = 1/16), then back
            gp = psA.tile([8, 8], F32, tag="gp")
            for t in range(4):
                nc.tensor.matmul(gp[:, 2 * t:2 * t + 2], lhsT=sel8_sb[:],
                                 rhs=chs[:, 2 * t:2 * t + 2], start=True, stop=True)
            gp_sb = pools.tile([8, 8], F32, tag="gpsb")
            nc.vector.tensor_scalar_mul(gp_sb[:], gp[:], float(NCORES))
            gx = psA.tile([128, 8], F32, tag="gx")
            for t in range(4):
                nc.tensor.matmul(gx[:, 2 * t:2 * t + 2], lhsT=selT_sb[:],
                                 rhs=gp_sb[:, 2 * t:2 * t + 2], start=True, stop=True)
            gxs = pools.tile([128, 8], F32, tag="gxs")
            nc.vector.tensor_copy(gxs[:], gx[:])
            gx3 = gxs.rearrange("p (t two) -> p t two", two=2)
            musq = pools.tile([128, 4], F32, tag="musq")
            nc.vector.tensor_tensor(out=musq[:], in0=gx3[:, :, 0], in1=gx3[:, :, 0],
                                    op=ALU.mult)
            var = pools.tile([128, 4], F32, tag="var")
            nc.vector.tensor_tensor(out=var[:], in0=gx3[:, :, 1], in1=musq[:],
                                    op=ALU.subtract)
            sd = pools.tile([128, 4], F32, tag="sd")
            nc.scalar.activation(out=sd[:], in_=var[:], func=AF.Sqrt,
                                 bias=eps_sb[:], scale=1.0)
            rstd = pools.tile([128, 4], F32, tag="rstd")
            nc.vector.reciprocal(out=rstd[:], in_=sd[:])
            nc.vector.tensor_tensor(out=A_sb[:], in0=rstd[:], in1=nw_sb[:],
                                    op=ALU.mult)
            muA = pools.tile([128, 4], F32, tag="muA")
            nc.vector.tensor_tensor(out=muA[:], in0=gx3[:, :, 0], in1=A_sb[:],
                                    op=ALU.mult)
            nc.vector.tensor_tensor(out=B_sb[:], in0=nb_sb[:], in1=muA[:],
                                    op=ALU.subtract)

            # ---------- phase C: normalize + SiLU(tanh form) + qkv ----------
            poolq2 = es_cd.enter_context(tc.tile_pool(name="poolq2", bufs=1))
            psB = es_b.enter_context(tc.tile_pool(name="psB", bufs=3, space="PSUM"))

            # u = A*x+B; sig = 0.5 + 0.5*tanh(u/2) == sigmoid(u); h = u*sig.
            # tanh is in exp_and_others: this also pre-loads the exp table.
            u_sb = poolq2.tile([128, 2048], F32)
            t_sb = poolq2.tile([128, 2048], F32)
            s_sb = poolq2.tile([128, 2048], F32)
            h_sb = poolq2.tile([128, 2048], F32R)
            for t in range(4):
                csl = slice(t * 512, (t + 1) * 512)
                eng = nc.vector if t % 2 == 0 else nc.gpsimd
                eng.tensor_scalar(out=u_sb[:, csl], in0=xblk_sb[:, csl],
                                  scalar1=A_sb[:, t:t + 1],
                                  scalar2=B_sb[:, t:t + 1],
                                  op0=ALU.mult, op1=ALU.add)
                nc.scalar.activation(out=t_sb[:, csl], in_=u_sb[:, csl],
                                     func=AF.Tanh, scale=0.5)
                eng.tensor_scalar(out=s_sb[:, csl], in0=t_sb[:, csl],
                                  scalar1=0.5, scalar2=0.5,
                                  op0=ALU.mult, op1=ALU.add)
                eng.tensor_tensor(out=h_sb[:, csl], in0=u_sb[:, csl],
                                  in1=s_sb[:, csl], op=ALU.mult)

            vs = poolq2.tile([128, 2048], BF16)
            # qkv order: k chunks, v chunks, then q -- K/Q mirrors + V
            # transposes all complete by the time S needs them.
            # q/k chunk 2t -> rows 0:64 cols 2t*512; 2t+1 -> rows 64:128.
            for ot in [4, 5, 6, 7, 8, 9, 10, 11, 0, 1, 2, 3]:
                ps = psB.tile([128, 512], F32, tag="qkvps")
                for kt in range(4):
                    nc.tensor.matmul(
                        ps[:],
                        lhsT=qkvw_sb[:, kt * 1536 + ot * 128:
                                     kt * 1536 + (ot + 1) * 128],
                        rhs=h_sb[:, kt * 512:(kt + 1) * 512],
                        start=(kt == 0), stop=(kt == 3))
                kind, t = ot // 4, ot % 4
                if kind == 2:
                    nc.vector.tensor_scalar_add(vs[:, t * 512:(t + 1) * 512], ps[:],
                                                qb_sb[:, ot:ot + 1])
                else:
                    dst = QT if kind == 0 else KT
                    nc.vector.tensor_scalar_add(
                        dst[0:64, (2 * t) * 512:(2 * t + 1) * 512],
                        ps[0:64, :], qb_sb[0:64, ot:ot + 1])
                    nc.vector.tensor_scalar_add(
                        dst[64:128, (2 * t + 1) * 512:(2 * t + 2) * 512],
                        ps[64:128, :], qb_sb[64:128, ot:ot + 1])
                if ot == 7 or ot == 3:
                    # mirror: even chunks (rows 0:64) up, odd chunks down,
                    # so both partition halves hold every chunk
                    dst = KT if ot == 7 else QT
                    d3 = dst.rearrange("p (u two n) -> p u two n", two=2, n=512)
                    nc.sync.dma_start(out=d3[64:128, :, 0, :], in_=d3[0:64, :, 0, :])
                    nc.sync.dma_start(out=d3[0:64, :, 1, :], in_=d3[64:128, :, 1, :])
                if ot == 11:
                    # ---------- phase D: Vp layout (PE transposes, bf16) ----
                    Vp3 = Vp.rearrange("p (j c) -> p j c", c=65)
                    nc.vector.memset(Vp3[:, :, 64:65], 1.0)
                    for tt in range(4):
                        for b in range(4):
                            pst = psB.tile([128, 128], BF16, tag="vtr")
                            nc.tensor.transpose(
                                pst[:],
                                in_=vs[:, tt * 512 + b * 128:tt * 512 + (b + 1) * 128],
                                identity=id_sb[:])
                            j1, j2 = 8 * tt + b, 8 * tt + 4 + b
                            nc.vector.tensor_copy(Vp3[:, j1, 0:64], pst[:, 0:64])
                            nc.vector.tensor_copy(Vp3[:, j2, 0:64], pst[:, 64:128])

        # ---------- phase E: attention (software-pipelined S/exp | O) ----------
        with ExitStack() as es_e:
            psS = es_e.enter_context(tc.tile_pool(name="psS", bufs=2, space="PSUM"))
            psO = es_e.enter_context(tc.tile_pool(name="psO", bufs=2, space="PSUM"))
            poolPB = es_e.enter_context(tc.tile_pool(name="poolPB", bufs=3))
            poolsm = es_e.enter_context(tc.tile_pool(name="poolsm", bufs=5))

            groups = [(j0, min(3, 32 - j0)) for j0 in range(0, 32, 3)]
            Vp3 = Vp.rearrange("p (j c) -> p j c", c=65)
            PBts, opss, OuSs, rDs = {}, {}, {}, {}

            def emit_drain_a(I):
                # evacuate O sums + reciprocal of the denominator row; the
                # dependent broadcast matmul runs one I later (emit_drain_b)
                # so the PE never waits on the reciprocal
                OuSs[I] = poolsm.tile([65, 512], F32, tag="OuS", name=f"OuS{I}")
                nc.vector.tensor_copy(OuSs[I][:], opss[I][:])
                rDf = poolsm.tile([1, 512], F32, tag="rDf", name=f"rDf{I}")
                nc.vector.reciprocal(out=rDf[:], in_=OuSs[I][64:65, :])
                rDs[I] = poolsm.tile([1, 512], F32R, tag="rD", name=f"rD{I}")
                nc.vector.tensor_copy(rDs[I][:], rDf[:])
                del opss[I], PBts[I]

            def emit_drain_b(I):
                u, par = I // 2, I % 2
                dps = psO.tile([64, 512], F32, tag="ops", name=f"dps{I}")
                nc.tensor.matmul(dps[:], lhsT=ones64_sb[:],
                                 rhs=rDs[I][:], start=True, stop=True)
                if par == 0:
                    nc.vector.tensor_tensor(out=ONorm[0:64, u * 512:(u + 1) * 512],
                                            in0=OuSs[I][0:64, :], in1=dps[:],
                                            op=ALU.mult)
                else:
                    stg = poolsm.tile([64, 512], F32, tag="stg", name=f"stg{I}")
                    nc.vector.tensor_tensor(out=stg[:], in0=OuSs[I][0:64, :],
                                            in1=dps[:], op=ALU.mult)
                    nc.gpsimd.dma_start(out=ONorm[64:128, u * 512:(u + 1) * 512],
                                        in_=stg[:])
                del OuSs[I], rDs[I]

            for I in range(9):
                if I < 8:
                    isl = slice(I * 512, (I + 1) * 512)
                    PBts[I] = poolPB.tile([128, 32 * 512], BF16, tag="PBt",
                                          name=f"PBt{I}")
                    opss[I] = psO.tile([65, 512], F32, tag="ops", name=f"ops{I}")
                for (j0, glen) in groups:
                    if I < 8:
                        sp = psS.tile([128, 1536], F32, tag="sp")
                        for jj in range(glen):
                            j = j0 + jj
                            r = slice(64, 128) if j % 2 else slice(0, 64)
                            nc.tensor.matmul(
                                sp[:, jj * 512:(jj + 1) * 512],
                                lhsT=KT[r, j * 128:(j + 1) * 128],
                                rhs=QT[r, isl],
                                start=True, stop=True)
                        nc.scalar.activation(
                            out=PBts[I][:, j0 * 512:(j0 + glen) * 512],
                            in_=sp[:, 0:glen * 512], func=AF.Exp, scale=SCALE)
                    if I > 0:
                        for jj in range(glen):
                            j = j0 + jj
                            nc.tensor.matmul(opss[I - 1][:], lhsT=Vp3[:, j, 0:65],
                                             rhs=PBts[I - 1][:, j * 512:(j + 1) * 512],
                                             start=(j == 0), stop=(j == 31))
                if I > 0:
                    emit_drain_a(I - 1)
                if I > 1:
                    emit_drain_b(I - 2)
            emit_drain_b(7)

        # ---------- phase F: proj (row-tiled) + bias + residual ----------
        with ExitStack() as es_f:
            psP = es_f.enter_context(tc.tile_pool(name="psP", bufs=4, space="PSUM"))
            poolf = es_f.enter_context(tc.tile_pool(name="poolf", bufs=2))
            for ot in range(4):
                ppA = psP.tile([128, 512], F32, tag="ppA")
                ppB = psP.tile([128, 512], F32, tag="ppB")
                for u in range(4):
                    nc.tensor.matmul(
                        ppA[:],
                        lhsT=pw_sb[0:64, u * 512 + ot * 128:u * 512 + (ot + 1) * 128],
                        rhs=ONorm[0:64, u * 512:(u + 1) * 512],
                        start=(u == 0), stop=(u == 3))
                for u in range(4):
                    nc.tensor.matmul(
                        ppB[:],
                        lhsT=pw_sb[64:128, u * 512 + ot * 128:u * 512 + (ot + 1) * 128],
                        rhs=ONorm[64:128, u * 512:(u + 1) * 512],
                        start=(u == 0), stop=(u == 3))
                fa = poolf.tile([128, 512], F32, tag="fa")
                nc.scalar.activation(out=fa[:], in_=ppA[:], func=AF.Identity,
                                     bias=pb_sb[:, ot:ot + 1], scale=1.0)
                fin = poolf.tile([128, 512], F32, tag="fin")
                nc.vector.tensor_tensor(out=fin[:], in0=fa[:], in1=ppB[:],
                                        op=ALU.add)
                nc.vector.tensor_tensor(out=fin[:], in0=fin[:],
                                        in1=xblk_sb[:, ot * 512:(ot + 1) * 512],
                                        op=ALU.add)
                nc.sync.dma_start(out=out[ot * 128:(ot + 1) * 128, :], in_=fin[:])


def _host_inputs(x, norm_w, norm_b, qkv_w, qkv_b, proj_w, proj_b):
    import ml_dtypes
    x2d = np.ascontiguousarray(np.asarray(x, np.float32).reshape(CH, N))
    qkv_w = np.asarray(qkv_w, np.float32)
    proj_w = np.asarray(proj_w, np.float32)
    # parity-split proj weights: chunk c -> rows 64*(c%2), cols (c//2)*512+o
    pw3 = proj_w.reshape(CH, 8, 64)          # [o, chunk, d']
    pw128 = np.zeros((128, 4 * CH), np.float32)
    for c in range(8):
        pw128[(c % 2) * 64:(c % 2) * 64 + 64, (c // 2) * CH:(c // 2 + 1) * CH] = \
            pw3[:, c, :].T
    common = {
        "xfb": np.ascontiguousarray(x2d.astype(ml_dtypes.bfloat16)),
        "qkvwT": np.ascontiguousarray(qkv_w.T),
        "qb": np.ascontiguousarray(np.asarray(qkv_b, np.float32).reshape(12, 128).T),
        "pw128": np.ascontiguousarray(pw128),
        "pb": np.ascontiguousarray(np.asarray(proj_b, np.float32).reshape(4, 128).T),
        "nw": np.ascontiguousarray(np.asarray(norm_w, np.float32).reshape(4, 128).T),
        "nbias": np.ascontiguousarray(np.asarray(norm_b, np.float32).reshape(4, 128).T),
        "identb": np.eye(128, dtype=ml_dtypes.bfloat16),
        "ones64": np.ones((1, 64), np.float32),
        "sel8": np.ascontiguousarray(
            (np.arange(128)[:, None] // GS == np.arange(8)[None, :])
            .astype(np.float32) / GS),
        "selT": np.ascontiguousarray(
            (np.arange(128)[None, :] // GS == np.arange(8)[:, None])
            .astype(np.float32) / NCORES),
    }
    in_maps = []
    for h in range(NCORES):
        m = dict(common)
        m["xblk"] = np.ascontiguousarray(x2d[:, h * NB:(h + 1) * NB])
        in_maps.append(m)
    return in_maps


_LAST_RESULT = {}


def kernel(x, norm_w, norm_b, qkv_w, qkv_b, proj_w, proj_b, _trace=False,
           _tmpdir=None):
    nc = _build()
    in_maps = _host_inputs(x, norm_w, norm_b, qkv_w, qkv_b, proj_w, proj_b)
    res = run_bass_kernel_spmd(nc, in_maps, core_ids=list(range(NCORES)),
                               trace=_trace, tmpdir=_tmpdir)
    _LAST_RESULT["res"] = res
    full = np.concatenate([res.results[h]["out"] for h in range(NCORES)], axis=1)
    return full.reshape(1, CH, 64, 64).astype(np.float32)


# revision 21
# speedup vs baseline: 1.1479x; 1.0867x over previous
"""AttentionBlock (GroupNorm+SiLU -> qkv -> 8-head attn -> proj -> residual)
on 8 TRN2 NeuronCores, head-parallel.

Key structure: the torch-faithful reshape q.transpose(1,2).reshape(B*NH,N,d)
makes "head" h = spatial positions n in [512h, 512h+512) -- attention is
block-diagonal over spatial blocks, so each core independently computes the
full pipeline for its block of 512 spatial positions and emits the final
output columns out[:, 512h:512h+512].

Sequence-axis permutation freedom (attention is equivariant under a common
permutation of Q/K/V rows) lets us use t = chunk*512 + n' ordering
(chunk = c//64, n' = spatial), which makes every layout a cheap copy.

Perf structure:
- GroupNorm stats split across DVE (bn_stats, chunks 0/1/3) and ACT
  (Square/Copy with accum_out, chunk 2), fed by a bf16 copy of x
  (half the DMA) while the core's own f32 block loads in parallel.
- DMA priority: x chunks first on the sync ring; qkv weights trigger on
  the scalar ring only after the stats ACTs, so stats aren't starved.
- SiLU computed as u*sigmoid(u) = 0.5u(1+tanh(u/2)): tanh lives in the
  exp table set, so the whole kernel needs only 2 ACT table loads and
  the exp set is resident before attention starts.
- S-matmuls (K=64) and proj (K=64) run 2x via PE row tiling: tiles
  (0,0)/(64,0) process even/odd blocks concurrently.  K^T/Q live
  duplicated on both partition halves (two strided mirror DMAs);
  ONorm/proj weights use a parity split.
- Softmax denominators ride the O-matmul (ones row appended to V);
  the per-I reciprocal is pipelined one I behind so the PE never waits.
- Softmax skips max-subtraction (scores*scale within [-0.76, 0.86]).
"""

import sys

if "/opt/trn_rl_repo" not in sys.path:
    sys.path.append("/opt/trn_rl_repo")  # fallback; the axon-site copy wins

import numpy as np

import concourse.bacc as bacc
import concourse.tile as tile
from concourse import mybir
from concourse.bass_utils import run_bass_kernel_spmd

F32 = mybir.dt.float32
F32R = mybir.dt.float32r
BF16 = mybir.dt.bfloat16
AF = mybir.ActivationFunctionType
ALU = mybir.AluOpType

CH = 512          # channels
N = 4096          # spatial positions (64*64)
NB = 512          # spatial block per core
NCORES = 8
G = 32            # groups
GS = 16           # channels per group
EPS = 1e-5
SCALE = 0.125     # d ** -0.5, d = 64


def _build():
    nc = bacc.Bacc(None, target_bir_lowering=False)

    xfb = nc.declare_dram_parameter("xfb", [CH, N], BF16, isOutput=False)
    xblk = nc.declare_dram_parameter("xblk", [CH, NB], F32, isOutput=False)
    qkvwT = nc.declare_dram_parameter("qkvwT", [CH, 3 * CH], F32R, isOutput=False)
    qb = nc.declare_dram_parameter("qb", [128, 12], F32, isOutput=False)
    pw128 = nc.declare_dram_parameter("pw128", [128, 4 * CH], F32R, isOutput=False)
    pb = nc.declare_dram_parameter("pb", [128, 4], F32, isOutput=False)
    nw = nc.declare_dram_parameter("nw", [128, 4], F32, isOutput=False)
    nbias = nc.declare_dram_parameter("nbias", [128, 4], F32, isOutput=False)
    identb = nc.declare_dram_parameter("identb", [128, 128], BF16, isOutput=False)
    ones64 = nc.declare_dram_parameter("ones64", [1, 64], F32R, isOutput=False)
    sel8 = nc.declare_dram_parameter("sel8", [128, 8], F32, isOutput=False)
    selT = nc.declare_dram_parameter("selT", [8, 128], F32, isOutput=False)
    out = nc.declare_dram_parameter("out", [CH, NB], F32, isOutput=True)

    with tile.TileContext(nc) as tc:
        _emit(nc, tc, locals())
    nc.finalize()
    return nc


def _emit(nc, tc, P):
    from contextlib import ExitStack

    xfb, xblk, qkvwT, qb, pw128, pb = (P[k] for k in
        ("xfb", "xblk", "qkvwT", "qb", "pw128", "pb"))
    nw, nbias, identb, ones64, sel8, selT, out = (P[k] for k in
        ("nw", "nbias", "identb", "ones64", "sel8", "selT", "out"))

    with ExitStack() as es:
        # ---------- persistent pools ----------
        persist = es.enter_context(tc.tile_pool(name="persist", bufs=1))
        consts = es.enter_context(tc.tile_pool(name="consts", bufs=1))

        xblk_sb = persist.tile([128, 4 * NB], F32)          # [p, t*512+n']
        pw_sb = persist.tile([128, 4 * CH], F32R)           # parity-split proj w
        QT = persist.tile([128, N], F32R)                   # both halves, all chunks
        KT = persist.tile([128, N], F32R)
        Vp = persist.tile([128, 32 * 65], BF16)             # [V_j | ones]
        ONorm = persist.tile([128, N // 2], F32R)           # parity-split attn out

        qb_sb = consts.tile([128, 12], F32)
        pb_sb = consts.tile([128, 4], F32)
        nw_sb = consts.tile([128, 4], F32)
        nb_sb = consts.tile([128, 4], F32)
        id_sb = consts.tile([128, 128], BF16)
        ones64_sb = consts.tile([1, 64], F32R)
        sel8_sb = consts.tile([128, 8], F32)
        selT_sb = consts.tile([8, 128], F32)
        eps_sb = consts.tile([128, 1], F32)
        A_sb = consts.tile([128, 4], F32)
        B_sb = consts.tile([128, 4], F32)

        # ---------- phase B: loads + GroupNorm stats (DVE || ACT) ----------
        with ExitStack() as es_b, ExitStack() as es_cd:
            pools = es_b.enter_context(tc.tile_pool(name="pools", bufs=4))
            poolbig = es_b.enter_context(tc.tile_pool(name="poolbig", bufs=1))
            psA = es_b.enter_context(tc.tile_pool(name="psA", bufs=1, space="PSUM"))

            # stats source: bf16 full x, highest priority, split across BOTH
            # hwdge rings in half-chunk pieces so stats can chase arrivals:
            # sync ring carries chunks 0,2 (DVE) and scalar carries 1,3 (ACT
            # front half / DVE back half)
            xfb_sb = poolbig.tile([128, 4 * N], BF16, tag="xfb")
            for t in (0, 2):
                for hh in range(2):
                    nc.sync.dma_start(
                        out=xfb_sb[:, t * N + hh * 2048:t * N + (hh + 1) * 2048],
                        in_=xfb[t * 128:(t + 1) * 128, hh * 2048:(hh + 1) * 2048])
            for t in (1, 3):
                for hh in range(2):
                    nc.scalar.dma_start(
                        out=xfb_sb[:, t * N + hh * 2048:t * N + (hh + 1) * 2048],
                        in_=xfb[t * 128:(t + 1) * 128, hh * 2048:(hh + 1) * 2048])
            for t in range(4):
                nc.scalar.dma_start(out=xblk_sb[:, t * NB:(t + 1) * NB],
                                    in_=xblk[t * 128:(t + 1) * 128, :])
            # consts on the gpsimd queue
            nc.gpsimd.dma_start(out=sel8_sb[:], in_=sel8[:])
            nc.gpsimd.dma_start(out=selT_sb[:], in_=selT[:])
            nc.gpsimd.dma_start(out=nw_sb[:], in_=nw[:])
            nc.gpsimd.dma_start(out=nb_sb[:], in_=nbias[:])
            nc.gpsimd.dma_start(out=qb_sb[:], in_=qb[:])
            nc.gpsimd.dma_start(out=pb_sb[:], in_=pb[:])
            nc.gpsimd.dma_start(out=id_sb[:], in_=identb[:])
            nc.gpsimd.dma_start(out=ones64_sb[:], in_=ones64[:])
            nc.vector.memset(eps_sb[:], EPS)

            # pin the ACT table set to sqrt_and_others (square/copy are in
            # every set) so stats + finalize need one load total
            junk1 = pools.tile([128, 1], F32, tag="junk1")
            nc.scalar.activation(out=junk1[:], in_=eps_sb[:], func=AF.Sqrt)

            chs = pools.tile([128, 8], F32, tag="chs")      # [mean_t, ex2_t]*4
            # chunk 1 + front half of chunk 3 on ACT (Square/Copy accumulate)
            sq = poolbig.tile([128, 2048], BF16, tag="sq")
            accs = {}
            for t, hh in ((1, 0), (1, 1), (3, 0)):
                a2 = pools.tile([128, 1], F32, tag="a2", name=f"a2_{t}_{hh}")
                a1 = pools.tile([128, 1], F32, tag="a1", name=f"a1_{t}_{hh}")
                src = xfb_sb[:, t * N + hh * 2048:t * N + (hh + 1) * 2048]
                nc.scalar.activation(out=sq[:], in_=src, func=AF.Square,
                                     accum_out=a2[:])
                nc.scalar.activation(out=sq[:], in_=src, func=AF.Copy,
                                     accum_out=a1[:])
                accs[(t, hh)] = (a1, a2)
                if t == 1 and hh == 1:
                    # qkv weights trigger here: after the stats loads, so
                    # they don't starve them, early enough for the qkv mms
                    qkvw_sb = es_cd.enter_context(
                        tc.tile_pool(name="poolq", bufs=1)).tile(
                            [128, 4 * 1536], F32R)
                    for kt in range(4):
                        nc.scalar.dma_start(
                            out=qkvw_sb[:, kt * 1536:(kt + 1) * 1536],
                            in_=qkvwT[kt * 128:(kt + 1) * 128, :])
                    nc.scalar.dma_start(out=pw_sb[:], in_=pw128[:])
            # chunk 1 combine
            s1 = pools.tile([128, 2], F32, tag="s1")
            nc.vector.tensor_tensor(out=s1[:, 0:1], in0=accs[(1, 0)][0][:],
                                    in1=accs[(1, 1)][0][:], op=ALU.add)
            nc.vector.tensor_tensor(out=s1[:, 1:2], in0=accs[(1, 0)][1][:],
                                    in1=accs[(1, 1)][1][:], op=ALU.add)
            nc.vector.tensor_scalar_mul(chs[:, 2:4], s1[:], 1.0 / N)

            # chunks 0, 2 fully + back half of chunk 3 on DVE bn_stats
            for t in (0, 2):
                st = pools.tile([128, 8, 6], F32, tag="st")
                for k in range(8):
                    nc.vector.bn_stats(out=st[:, k, :],
                                       in_=xfb_sb[:, t * N + k * 512:
                                                  t * N + (k + 1) * 512])
                mv = pools.tile([128, 2], F32, tag="mv")
                nc.vector.bn_aggr(out=mv[:], in_=st[:])
                nc.vector.tensor_copy(chs[:, 2 * t:2 * t + 1], mv[:, 0:1])
                msq = pools.tile([128, 1], F32, tag="msq")
                nc.vector.tensor_tensor(out=msq[:], in0=mv[:, 0:1], in1=mv[:, 0:1],
                                        op=ALU.mult)
                nc.vector.tensor_tensor(out=chs[:, 2 * t + 1:2 * t + 2],
                                        in0=msq[:], in1=mv[:, 1:2],
                                        op=ALU.add)
            st3 = pools.tile([128, 4, 6], F32, tag="st3")
            for k in range(4, 8):
                nc.vector.bn_stats(out=st3[:, k - 4, :],
                                   in_=xfb_sb[:, 3 * N + k * 512:
                                              3 * N + (k + 1) * 512])
            mv3 = pools.tile([128, 2], F32, tag="mv3")
            nc.vector.bn_aggr(out=mv3[:], in_=st3[:])
            # chunk 3 combine: 0.5*(front_sum/2048 + back_mean) etc.
            c3t = pools.tile([128, 2], F32, tag="c3t")
            nc.vector.tensor_scalar_mul(c3t[:, 0:1], accs[(3, 0)][0][:], 1.0 / 2048)
            nc.vector.tensor_scalar_mul(c3t[:, 1:2], accs[(3, 0)][1][:], 1.0 / 2048)
            e2b = pools.tile([128, 2], F32, tag="e2b")
            nc.vector.tensor_copy(e2b[:, 0:1], mv3[:, 0:1])
            msq3 = pools.tile([128, 1], F32, tag="msq3")
            nc.vector.tensor_tensor(out=msq3[:], in0=mv3[:, 0:1], in1=mv3[:, 0:1],
                                    op=ALU.mult)
            nc.vector.tensor_tensor(out=e2b[:, 1:2], in0=msq3[:], in1=mv3[:, 1:2],
                                    op=ALU.add)
            c3s = pools.tile([128, 2], F32, tag="c3s")
            nc.vector.tensor_tensor(out=c3s[:], in0=c3t[:], in1=e2b[:],
                                    op=ALU.add)
            nc.vector.tensor_scalar_mul(chs[:, 6:8], c3s[:], 0.5)

            # group fold: per-channel -> per-group (sel8 = 1/16), then back
            gp = psA.tile([8, 8], F32, tag="gp")
            for t in range(4):
                nc.tensor.matmul(gp[:, 2 * t:2 * t + 2], lhsT=sel8_sb[:],
                                 rhs=chs[:, 2 * t:2 * t + 2], start=True, stop=True)
            gp_sb = pools.tile([8, 8], F32, tag="gpsb")
            nc.vector.tensor_scalar_mul(gp_sb[:], gp[:], float(NCORES))
            gx = psA.tile([128, 8], F32, tag="gx")
            for t in range(4):
                nc.tensor.matmul(gx[:, 2 * t:2 * t + 2], lhsT=selT_sb[:],
                                 rhs=gp_sb[:, 2 * t:2 * t + 2], start=True, stop=True)
            gxs = pools.tile([128, 8], F32, tag="gxs")
            nc.vector.tensor_copy(gxs[:], gx[:])
            gx3 = gxs.rearrange("p (t two) -> p t two", two=2)
            musq = pools.tile([128, 4], F32, tag="musq")
            nc.vector.tensor_tensor(out=musq[:], in0=gx3[:, :, 0], in1=gx3[:, :, 0],
                                    op=ALU.mult)
            var = pools.tile([128, 4], F32, tag="var")
            nc.vector.tensor_tensor(out=var[:], in0=gx3[:, :, 1], in1=musq[:],
                                    op=ALU.subtract)
            sd = pools.tile([128, 4], F32, tag="sd")
            nc.scalar.activation(out=sd[:], in_=var[:], func=AF.Sqrt,
                                 bias=eps_sb[:], scale=1.0)
            rstd = pools.tile([128, 4], F32, tag="rstd")
            nc.vector.reciprocal(out=rstd[:], in_=sd[:])
            nc.vector.tensor_tensor(out=A_sb[:], in0=rstd[:], in1=nw_sb[:],
                                    op=ALU.mult)
            muA = pools.tile([128, 4], F32, tag="muA")
            nc.vector.tensor_tensor(out=muA[:], in0=gx3[:, :, 0], in1=A_sb[:],
                                    op=ALU.mult)
            nc.vector.tensor_tensor(out=B_sb[:], in0=nb_sb[:], in1=muA[:],
                                    op=ALU.subtract)

            # ---------- phase C: normalize + SiLU(tanh form) + qkv ----------
            poolq2 = es_cd.enter_context(tc.tile_pool(name="poolq2", bufs=1))
            psB = es_b.enter_context(tc.tile_pool(name="psB", bufs=3, space="PSUM"))

            # u = A*x+B; sig = 0.5 + 0.5*tanh(u/2) == sigmoid(u); h = u*sig.
            # tanh is in exp_and_others: this also pre-loads the exp table.
            u_sb = poolq2.tile([128, 2048], F32)
            t_sb = poolq2.tile([128, 2048], F32)
            s_sb = poolq2.tile([128, 2048], F32)
            h_sb = poolq2.tile([128, 2048], F32R)
            for t in range(4):
                csl = slice(t * 512, (t + 1) * 512)
                eng = nc.vector if t % 2 == 0 else nc.gpsimd
                eng.tensor_scalar(out=u_sb[:, csl], in0=xblk_sb[:, csl],
                                  scalar1=A_sb[:, t:t + 1],
                                  scalar2=B_sb[:, t:t + 1],
                                  op0=ALU.mult, op1=ALU.add)
                nc.scalar.activation(out=t_sb[:, csl], in_=u_sb[:, csl],
                                     func=AF.Tanh, scale=0.5)
                eng.tensor_scalar(out=s_sb[:, csl], in0=t_sb[:, csl],
                                  scalar1=0.5, scalar2=0.5,
                                  op0=ALU.mult, op1=ALU.add)
                eng.tensor_tensor(out=h_sb[:, csl], in0=u_sb[:, csl],
                                  in1=s_sb[:, csl], op=ALU.mult)

            vs = poolq2.tile([128, 2048], BF16)
            # qkv order: k chunks, v chunks, then q -- K/Q mirrors + V
            # transposes all complete by the time S needs them.
            # q/k chunk 2t -> rows 0:64 cols 2t*512; 2t+1 -> rows 64:128.
            for ot in [4, 5, 6, 7, 8, 9, 10, 11, 0, 1, 2, 3]:
                ps = psB.tile([128, 512], F32, tag="qkvps")
                for kt in range(4):
                    nc.tensor.matmul(
                        ps[:],
                        lhsT=qkvw_sb[:, kt * 1536 + ot * 128:
                                     kt * 1536 + (ot + 1) * 128],
                        rhs=h_sb[:, kt * 512:(kt + 1) * 512],
                        start=(kt == 0), stop=(kt == 3))
                kind, t = ot // 4, ot % 4
                if kind == 2:
                    nc.vector.tensor_scalar_add(vs[:, t * 512:(t + 1) * 512], ps[:],
                                                qb_sb[:, ot:ot + 1])
                else:
                    dst = QT if kind == 0 else KT
                    nc.vector.tensor_scalar_add(
                        dst[0:64, (2 * t) * 512:(2 * t + 1) * 512],
                        ps[0:64, :], qb_sb[0:64, ot:ot + 1])
                    nc.vector.tensor_scalar_add(
                        dst[64:128, (2 * t + 1) * 512:(2 * t + 2) * 512],
                        ps[64:128, :], qb_sb[64:128, ot:ot + 1])
                if ot == 7 or ot == 3:
                    # mirror: even chunks (rows 0:64) up, odd chunks down,
                    # so both partition halves hold every chunk
                    dst = KT if ot == 7 else QT
                    d3 = dst.rearrange("p (u two n) -> p u two n", two=2, n=512)
                    nc.sync.dma_start(out=d3[64:128, :, 0, :], in_=d3[0:64, :, 0, :])
                    nc.sync.dma_start(out=d3[0:64, :, 1, :], in_=d3[64:128, :, 1, :])
                if ot == 11:
                    # ---------- phase D: Vp layout (PE transposes, bf16) ----
                    Vp3 = Vp.rearrange("p (j c) -> p j c", c=65)
                    nc.vector.memset(Vp3[:, :, 64:65], 1.0)
                    for tt in range(4):
                        for b in range(4):
                            pst = psB.tile([128, 128], BF16, tag="vtr")
                            nc.tensor.transpose(
                                pst[:],
                                in_=vs[:, tt * 512 + b * 128:tt * 512 + (b + 1) * 128],
                                identity=id_sb[:])
                            j1, j2 = 8 * tt + b, 8 * tt + 4 + b
                            nc.vector.tensor_copy(Vp3[:, j1, 0:64], pst[:, 0:64])
                            nc.vector.tensor_copy(Vp3[:, j2, 0:64], pst[:, 64:128])

        # ---------- phase E: attention (software-pipelined S/exp | O) ----------
        with ExitStack() as es_e:
            psS = es_e.enter_context(tc.tile_pool(name="psS", bufs=2, space="PSUM"))
            psO = es_e.enter_context(tc.tile_pool(name="psO", bufs=2, space="PSUM"))
            poolPB = es_e.enter_context(tc.tile_pool(name="poolPB", bufs=3))
            poolsm = es_e.enter_context(tc.tile_pool(name="poolsm", bufs=5))

            groups = [(j0, min(3, 32 - j0)) for j0 in range(0, 32, 3)]
            Vp3 = Vp.rearrange("p (j c) -> p j c", c=65)
            PBts, opss, OuSs = {}, {}, {}

            def emit_drain(I):
                # evacuate O sums, invert the denominator row, replicate it
                # across 64 partitions (DMA broadcast), multiply on gpsimd --
                # the PE is never involved, so it cannot stall on this chain
                u, par = I // 2, I % 2
                OuS = poolsm.tile([65, 512], F32, tag="OuS", name=f"OuS{I}")
                nc.vector.tensor_copy(OuS[:], opss[I][:])
                rDf = poolsm.tile([1, 512], F32, tag="rDf", name=f"rDf{I}")
                nc.vector.reciprocal(out=rDf[:], in_=OuS[64:65, :])
                rB = poolsm.tile([64, 512], F32, tag="rB", name=f"rB{I}")
                nc.gpsimd.partition_broadcast(rB[:], rDf[:])
                if par == 0:
                    nc.gpsimd.tensor_tensor(out=ONorm[0:64, u * 512:(u + 1) * 512],
                                            in0=OuS[0:64, :], in1=rB[:],
                                            op=ALU.mult)
                else:
                    stg = poolsm.tile([64, 512], F32, tag="stg", name=f"stg{I}")
                    nc.gpsimd.tensor_tensor(out=stg[:], in0=OuS[0:64, :],
                                            in1=rB[:], op=ALU.mult)
                    nc.gpsimd.dma_start(out=ONorm[64:128, u * 512:(u + 1) * 512],
                                        in_=stg[:])
                del opss[I], PBts[I]

            for I in range(9):
                if I < 8:
                    isl = slice(I * 512, (I + 1) * 512)
                    PBts[I] = poolPB.tile([128, 32 * 512], BF16, tag="PBt",
                                          name=f"PBt{I}")
                    opss[I] = psO.tile([65, 512], F32, tag="ops", name=f"ops{I}")
                for (j0, glen) in groups:
                    if I < 8:
                        sp = psS.tile([128, 1536], F32, tag="sp")
                        for jj in range(glen):
                            j = j0 + jj
                            r = slice(64, 128) if j % 2 else slice(0, 64)
                            nc.tensor.matmul(
                                sp[:, jj * 512:(jj + 1) * 512],
                                lhsT=KT[r, j * 128:(j + 1) * 128],
                                rhs=QT[r, isl],
                                start=True, stop=True)
                        nc.scalar.activation(
                            out=PBts[I][:, j0 * 512:(j0 + glen) * 512],
                            in_=sp[:, 0:glen * 512], func=AF.Exp, scale=SCALE)
                    if I > 0:
                        for jj in range(glen):
                            j = j0 + jj
                            nc.tensor.matmul(opss[I - 1][:], lhsT=Vp3[:, j, 0:65],
                                             rhs=PBts[I - 1][:, j * 512:(j + 1) * 512],
                                             start=(j == 0), stop=(j == 31))
                if I > 0:
                    emit_drain(I - 1)

        # ---------- phase F: proj (row-tiled) + bias + residual ----------
        with ExitStack() as es_f:
            psP = es_f.enter_context(tc.tile_pool(name="psP", bufs=4, space="PSUM"))
            poolf = es_f.enter_context(tc.tile_pool(name="poolf", bufs=2))
            for ot in range(4):
                ppA = psP.tile([128, 512], F32, tag="ppA")
                ppB = psP.tile([128, 512], F32, tag="ppB")
                for u in range(4):
                    nc.tensor.matmul(
                        ppA[:],
                        lhsT=pw_sb[0:64, u * 512 + ot * 128:u * 512 + (ot + 1) * 128],
                        rhs=ONorm[0:64, u * 512:(u + 1) * 512],
                        start=(u == 0), stop=(u == 3))
                for u in range(4):
                    nc.tensor.matmul(
                        ppB[:],
                        lhsT=pw_sb[64:128, u * 512 + ot * 128:u * 512 + (ot + 1) * 128],
                        rhs=ONorm[64:128, u * 512:(u + 1) * 512],
                        start=(u == 0), stop=(u == 3))
                fa = poolf.tile([128, 512], F32, tag="fa")
                nc.scalar.activation(out=fa[:], in_=ppA[:], func=AF.Identity,
                                     bias=pb_sb[:, ot:ot + 1], scale=1.0)
                fin = poolf.tile([128, 512], F32, tag="fin")
                nc.vector.tensor_tensor(out=fin[:], in0=fa[:], in1=ppB[:],
                                        op=ALU.add)
                nc.vector.tensor_tensor(out=fin[:], in0=fin[:],
                                        in1=xblk_sb[:, ot * 512:(ot + 1) * 512],
                                        op=ALU.add)
                nc.sync.dma_start(out=out[ot * 128:(ot + 1) * 128, :], in_=fin[:])


def _host_inputs(x, norm_w, norm_b, qkv_w, qkv_b, proj_w, proj_b):
    import ml_dtypes
    x2d = np.ascontiguousarray(np.asarray(x, np.float32).reshape(CH, N))
    qkv_w = np.asarray(qkv_w, np.float32)
    proj_w = np.asarray(proj_w, np.float32)
    # parity-split proj weights: chunk c -> rows 64*(c%2), cols (c//2)*512+o
    pw3 = proj_w.reshape(CH, 8, 64)          # [o, chunk, d']
    pw128 = np.zeros((128, 4 * CH), np.float32)
    for c in range(8):
        pw128[(c % 2) * 64:(c % 2) * 64 + 64, (c // 2) * CH:(c // 2 + 1) * CH] = \
            pw3[:, c, :].T
    common = {
        "xfb": np.ascontiguousarray(x2d.astype(ml_dtypes.bfloat16)),
        "qkvwT": np.ascontiguousarray(qkv_w.T),
        "qb": np.ascontiguousarray(np.asarray(qkv_b, np.float32).reshape(12, 128).T),
        "pw128": np.ascontiguousarray(pw128),
        "pb": np.ascontiguousarray(np.asarray(proj_b, np.float32).reshape(4, 128).T),
        "nw": np.ascontiguousarray(np.asarray(norm_w, np.float32).reshape(4, 128).T),
        "nbias": np.ascontiguousarray(np.asarray(norm_b, np.float32).reshape(4, 128).T),
        "identb": np.eye(128, dtype=ml_dtypes.bfloat16),
        "ones64": np.ones((1, 64), np.float32),
        "sel8": np.ascontiguousarray(
            (np.arange(128)[:, None] // GS == np.arange(8)[None, :])
            .astype(np.float32) / GS),
        "selT": np.ascontiguousarray(
            (np.arange(128)[None, :] // GS == np.arange(8)[:, None])
            .astype(np.float32) / NCORES),
    }
    in_maps = []
    for h in range(NCORES):
        m = dict(common)
        m["xblk"] = np.ascontiguousarray(x2d[:, h * NB:(h + 1) * NB])
        in_maps.append(m)
    return in_maps


_LAST_RESULT = {}


def kernel(x, norm_w, norm_b, qkv_w, qkv_b, proj_w, proj_b, _trace=False,
           _tmpdir=None):
    nc = _build()
    in_maps = _host_inputs(x, norm_w, norm_b, qkv_w, qkv_b, proj_w, proj_b)
    res = run_bass_kernel_spmd(nc, in_maps, core_ids=list(range(NCORES)),
                               trace=_trace, tmpdir=_tmpdir)
    _LAST_RESULT["res"] = res
    full = np.concatenate([res.results[h]["out"] for h in range(NCORES)], axis=1)
    return full.reshape(1, CH, 64, 64).astype(np.float32)
